# revision 2
# baseline (speedup 1.0000x reference)
"""DeepSSM Trainium2 kernel v3 (8 NeuronCores, data-parallel over batch).

Same math as v2 (conv collapsed to S_inf + tail correction, centered pre-LN
stream) but rebalanced across all four compute engines against the cost
model:

- The residual identity matmul is gone: the PSUM join is now a single
  DVE/Pool scalar_tensor_tensor  Y = (ps + pbt) + t  (PE -1.7us/layer).
- rsqrt is ACT Identity(S+eps) -> DVE reciprocal_approx_fast -> ACT Sqrt
  (3 small ops, ~0.9us/layer cheaper than the 6-op DVE bit-trick).
- sq and t' use TensorScalarPtr (STT with bypass) which the DVE runs in
  4x mode on all-SBUF fp16 operands (193ns vs 327ns per [128,512]).
- Per-chunk engine assignment (join/sq/evac/t') is a CONFIG sweep: part
  of the join+evac load rides the otherwise-idle Pool (gpsimd) engine.
- Exit-layer pooled sums ride the t' STT's accum_out for free.
"""

import numpy as np

D_MODEL = 256
N_LAYERS = 8
NUM_CLASSES = 3
BATCH = 8
SEQ = 2048
JW = 256
TAIL = 256
LN_EPS = 1e-5
EXIT_LAYERS = (1, 3, 5, 7)
NT = 4
NTW = SEQ // NT  # 512
NSUB = 16  # 128-position sub-chunks

_CACHE = {}

# per-nt engine assignment (sweepable):
#   join_eng: 'd' DVE STT (ps+pbt)+t from PSUM; 'p' same on Pool;
#             'a' ACT Identity evac (ps+pbt) then DVE STT add t (4x)
#   sq_eng:   'd' DVE STT Y*Y (4x); 'a' ACT Square(Y); 'p' Pool TT
#   evac_eng: 'a' ACT Copy ib->i16; 'p' Pool TS; 'D' no evac, t' STT reads PSUM
CONFIG = dict(
    join_eng=("d", "d", "p", "p"),
    sq_eng=("d", "d", "d", "d"),
    evac_eng=("a", "a", "p", "p"),
    gelu_pieces=((0, 512), (512, 1024), (1024, 1792)),
    rsqrt_pairs=True,
    pw_bufs=4,
    h0_late=True,
)


def _host_prep(inputs):
    f64 = np.float64
    A = 1.0 / (1.0 + np.exp(-inputs["A_params"].astype(f64)))  # [nl, d]
    lnA = np.log(A)
    CB = inputs["C_params"].astype(f64) * inputs["B_params"].astype(f64)
    j1 = np.arange(JW, dtype=f64)
    lt = (TAIL - 1.0) - np.arange(TAIL, dtype=f64)
    W1 = np.exp(lnA[:, :, None] * j1[None, None, :])            # [nl, d, JW]
    Wt = CB[:, :, None] * np.exp(lnA[:, :, None] * lt[None, None, :])

    def to_chunks(T, dt):  # [nl, d, l] -> [128, nl, 2, l]
        return np.ascontiguousarray(
            T.reshape(N_LAYERS, 2, 128, -1).transpose(2, 0, 1, 3)
        ).astype(dt)

    pW = inputs["proj_W"].astype(f64)                            # [nl, do, di]
    pWc = pW - pW.mean(axis=1, keepdims=True)
    PtT_all = np.ascontiguousarray(
        pWc.transpose(0, 2, 1).reshape(N_LAYERS, 2, 128, D_MODEL).transpose(2, 0, 1, 3)
    ).astype(np.float16)                                          # [128,nl,2,256]

    Dp_all = np.ascontiguousarray(
        inputs["D_params"].reshape(N_LAYERS, 2, 128).transpose(2, 0, 1)
    ).astype(np.float32)
    pb = inputs["proj_b"].astype(f64)
    pbt = pb - pb.mean(axis=1, keepdims=True)
    pbt_all = np.ascontiguousarray(
        pbt.reshape(N_LAYERS, 2, 128).transpose(2, 0, 1)
    ).astype(np.float32)

    # layer-0 stream is RAW h0 = inW*x + in_b; its centering for the LN
    # stats rides in via corrW (x-dependent) and the pbt[0] constant.
    inW = inputs["in_W"][:, 0].astype(f64)
    inb = inputs["in_b"].astype(f64)
    corrW_row = np.full((1, D_MODEL), -inW.mean(), dtype=np.float16)
    pbt_all[:, 0, :] -= np.float32(inb.mean())

    hW = inputs["head_W"].astype(f64) / SEQ                      # [4, nc, d]
    headWT_all = np.ascontiguousarray(
        hW.transpose(2, 0, 1).reshape(2, 128, 4, NUM_CLASSES).transpose(1, 0, 2, 3)
    ).astype(np.float32)                                          # [128,2,4,3]
    headb_all = np.ascontiguousarray(
        inputs["head_b"].astype(np.float32).T.reshape(NUM_CLASSES, 4)
    )

    sel = np.zeros((8, NSUB * 128), np.float16)
    for g in range(NSUB):
        sel[g % 8, g * 128:(g + 1) * 128] = 1.0

    return dict(
        W1_all=to_chunks(W1, np.float16),
        Wt_all=to_chunks(Wt, np.float16),
        PtT_all=PtT_all,
        Dp_all=Dp_all,
        pbt_all=pbt_all,
        corrW_row=corrW_row,
        sel_all=sel,
        onesI_in=np.ascontiguousarray(
            np.tile(np.eye(8, dtype=np.float16)[None] / D_MODEL, (128, 1, 1))
        ),
        headWT_all=headWT_all,
        headb_all=headb_all,
    )


def _split_drain_waits(nc, mybir, maxw=1):
    """Hoist excess sync waits onto same-engine NOPs (walrus ISA limit)."""
    for f in nc.m.functions:
        for blk in f.blocks:
            insts = list(blk.instructions)
            changed = False
            new_list = []
            for ins in insts:
                w = (
                    list(ins.sync_info.on_wait)
                    if ins.sync_info and ins.sync_info.on_wait
                    else []
                )
                if len(w) > maxw:
                    changed = True
                    extra, keep = w[:-maxw], w[-maxw:]
                    for j in range(0, len(extra), maxw):
                        nop = mybir.InstNoOp(
                            name=f"{ins.name}-wsplit{j}", ins=[], outs=[]
                        )
                        nop.engine = ins.engine
                        nop.sync_info = mybir.SyncInfo(
                            on_wait=extra[j : j + maxw], on_update=[]
                        )
                        new_list.append(nop)
                    ins.sync_info.on_wait = keep
                new_list.append(ins)
            if changed:
                blk.instructions = new_list


def _build_nc(sim_safe=False, split=True, cfg=None):
    import concourse.bass as bass
    import concourse.tile as tile
    import concourse.mybir as mybir

    F32 = mybir.dt.float32
    F16 = mybir.dt.float16
    OP = mybir.AluOpType
    ACTF = mybir.ActivationFunctionType
    GELU = ACTF.Sigmoid if sim_safe else ACTF.Gelu
    if cfg is None:
        cfg = CONFIG

    nc = bass.Bass("TRN2", target_bir_lowering=False, debug=False)

    d_h0 = nc.dram_tensor("h0_in", [128, 2, SEQ], F16, kind="ExternalInput")
    d_W1 = nc.dram_tensor("W1_all", [128, N_LAYERS, 2, JW], F16, kind="ExternalInput")
    d_Wt = nc.dram_tensor("Wt_all", [128, N_LAYERS, 2, TAIL], F16, kind="ExternalInput")
    d_Pt = nc.dram_tensor("PtT_all", [128, N_LAYERS, 2, D_MODEL], F16, kind="ExternalInput")
    d_Dp = nc.dram_tensor("Dp_all", [128, N_LAYERS, 2], F32, kind="ExternalInput")
    d_pbt = nc.dram_tensor("pbt_all", [128, N_LAYERS, 2], F32, kind="ExternalInput")
    d_corrW = nc.dram_tensor("corrW_row", [1, D_MODEL], F16, kind="ExternalInput")
    d_x16 = nc.dram_tensor("x_row16", [1, SEQ], F16, kind="ExternalInput")
    d_sel = nc.dram_tensor("sel_all", [8, NSUB * 128], F16, kind="ExternalInput")
    d_onesI = nc.dram_tensor("onesI_in", [128, 8, 8], F16, kind="ExternalInput")
    d_hW = nc.dram_tensor("headWT_all", [128, 2, 4, NUM_CLASSES], F32, kind="ExternalInput")
    d_hb = nc.dram_tensor("headb_all", [NUM_CLASSES, 4], F32, kind="ExternalInput")
    d_out = nc.dram_tensor("logits_out", [NUM_CLASSES, 4], F32, kind="ExternalOutput")

    with tile.TileContext(nc) as tc:
        from contextlib import ExitStack

        ctx = ExitStack()
        with ctx:
            const = ctx.enter_context(tc.tile_pool(name="const", bufs=1))
            stream = ctx.enter_context(tc.tile_pool(name="stream", bufs=cfg.get("stream_bufs", 3)))
            ypool = ctx.enter_context(tc.tile_pool(name="ypool", bufs=cfg.get("y_bufs", 2)))
            vpool = ctx.enter_context(tc.tile_pool(name="vpool", bufs=cfg.get("v_bufs", 2)))
            sqpool = ctx.enter_context(tc.tile_pool(name="sqpool", bufs=cfg.get("sq_bufs", 2)))
            epool = ctx.enter_context(tc.tile_pool(name="epool", bufs=cfg.get("e_bufs", 2)))
            small = ctx.enter_context(tc.tile_pool(name="small", bufs=cfg.get("small_bufs", 2)))
            stat = ctx.enter_context(tc.tile_pool(name="stat", bufs=cfg.get("stat_bufs", 2)))
            pacc = ctx.enter_context(tc.tile_pool(name="pacc", bufs=5))
            pw = ctx.enter_context(tc.tile_pool(name="pw", bufs=cfg.get("pw_bufs", 4), space="PSUM"))
            pstat = ctx.enter_context(tc.tile_pool(name="pstat", bufs=1, space="PSUM"))
            pinv = ctx.enter_context(tc.tile_pool(name="pinv", bufs=2, space="PSUM"))

            # ---- constants / weights to SBUF ----
            # SP issues DMAs in-order: layer-0's working set first, cold
            # layers and head tables last.
            t = stream.tile([128, 2, SEQ], F16, tag="stream")
            nc.sync.dma_start(out=t[:, :, 0:NTW], in_=d_h0.ap()[:, :, 0:NTW])
            Dp_sb = const.tile([128, N_LAYERS, 2], F32)
            nc.sync.dma_start(out=Dp_sb[:], in_=d_Dp.ap())
            Pt_sb = const.tile([128, N_LAYERS, 2, D_MODEL], F16)
            W1_sb = const.tile([128, N_LAYERS, 2, JW], F16)
            Wt_sb = const.tile([128, N_LAYERS, 2, TAIL], F16)
            nc.sync.dma_start(out=W1_sb[:, 0], in_=d_W1.ap()[:, 0])
            nc.sync.dma_start(out=t[:, :, NTW : 2 * NTW], in_=d_h0.ap()[:, :, NTW : 2 * NTW])
            nc.sync.dma_start(out=Wt_sb[:, 0], in_=d_Wt.ap()[:, 0])
            if cfg.get("h0_late"):
                nc.sync.dma_start(out=t[:, :, 2 * NTW : 3 * NTW], in_=d_h0.ap()[:, :, 2 * NTW : 3 * NTW])
                nc.sync.dma_start(out=Pt_sb[:, 0], in_=d_Pt.ap()[:, 0])
                nc.sync.dma_start(out=t[:, :, 3 * NTW :], in_=d_h0.ap()[:, :, 3 * NTW :])
            else:
                nc.sync.dma_start(out=Pt_sb[:, 0], in_=d_Pt.ap()[:, 0])
                nc.sync.dma_start(out=t[:, :, 2 * NTW : 3 * NTW], in_=d_h0.ap()[:, :, 2 * NTW : 3 * NTW])
                nc.sync.dma_start(out=t[:, :, 3 * NTW :], in_=d_h0.ap()[:, :, 3 * NTW :])
            pbt_sb = const.tile([128, N_LAYERS, 2], F32)
            nc.sync.dma_start(out=pbt_sb[:], in_=d_pbt.ap())
            x16_row = const.tile([1, SEQ], F16)
            nc.sync.dma_start(out=x16_row[:], in_=d_x16.ap())
            corrW_row = const.tile([1, D_MODEL], F16)
            nc.sync.dma_start(out=corrW_row[:], in_=d_corrW.ap())
            onesI_sb = const.tile([128, 8, 8], F16)
            nc.sync.dma_start(out=onesI_sb[:], in_=d_onesI.ap())
            sel_sb = const.tile([8, NSUB * 128], F16)
            nc.sync.dma_start(out=sel_sb[:], in_=d_sel.ap())
            for i in range(1, N_LAYERS):
                nc.sync.dma_start(out=W1_sb[:, i], in_=d_W1.ap()[:, i])
                nc.sync.dma_start(out=Wt_sb[:, i], in_=d_Wt.ap()[:, i])
                nc.sync.dma_start(out=Pt_sb[:, i], in_=d_Pt.ap()[:, i])
            hW_sb = const.tile([128, 2, 4, NUM_CLASSES], F32)
            nc.sync.dma_start(out=hW_sb[:], in_=d_hW.ap())
            hb_sb = const.tile([NUM_CLASSES, 4], F32)
            nc.sync.dma_start(out=hb_sb[:], in_=d_hb.ap())
            eps_col = const.tile([8, 1], F32)
            nc.vector.memset(eps_col, LN_EPS)

            logits_sb = const.tile([NUM_CLASSES, 4], F32)
            pooled = {}

            # ---------------- shared tail: Y -> t_next ----------------
            def tail(Y, sq, li, exit_idx):
                """Y fp16 [128,2,SEQ] (centered pre-LN) -> t fp16 (normalized).

                stats (tiny PE matmuls into per-pair PSUM stat tiles) ->
                rsqrt (ACT eps-evac, DVE reciprocal_approx_fast, ACT Sqrt)
                -> PE selector-broadcast -> evac -> t' STT (4x, accum_out
                doubles as exit-layer pooled partial sums)."""
                S_psA = pstat.tile([8, 128], F32, tag="pstat")
                S_psB = pstat.tile([8, 128], F32, tag="pstatB")
                S_psp = {0: S_psA, 1: S_psB}
                t_new = stream.tile([128, 2, SEQ], F16, tag="stream")
                r16p = {}
                pc = None
                if exit_idx is not None:
                    pc = pacc.tile([128, 2, NT], F32, tag="pacc")
                    pooled[exit_idx] = pc

                def stats_chunk(nt):
                    for c in range(4):
                        g = nt * 4 + c
                        csl = slice(g * 128, (g + 1) * 128)
                        for m in range(2):
                            nc.tensor.matmul(
                                S_psp[g // 8][:],
                                lhsT=onesI_sb[:, g % 8, :],
                                rhs=sq[:, m, csl],
                                start=(g % 8 == 0 and m == 0),
                                stop=(g % 8 == 7 and m == 1),
                            )

                def rsqrt_pair(p):
                    # inv = 1/sqrt(S+eps): ACT evacs S with the eps bias,
                    # DVE does the fp32 reciprocal (custom op, ~18 bits),
                    # ACT takes the square root straight to fp16.
                    v16 = stat.tile([8, 128], F32, tag="v16")
                    y32 = stat.tile([8, 128], F32, tag="y32")
                    r16 = stat.tile([8, 128], F16, tag="r16")
                    r16p[p] = r16
                    nc.scalar.activation(
                        out=v16[:], in_=S_psp[p][:], func=ACTF.Identity,
                        bias=eps_col[:], scale=1.0,
                    )
                    nc.vector.reciprocal_approx_fast(out=y32[:], in_=v16[:])
                    nc.scalar.activation(
                        out=r16[:], in_=y32[:], func=ACTF.Sqrt,
                        bias=0.0, scale=1.0,
                    )

                def bcast_tp(nt):
                    # reads only this nt-pair's r16 tile so pair-0 broadcasts
                    # don't wait on pair-1's rsqrt
                    sl = slice(nt * NTW, (nt + 1) * NTW)
                    r16 = r16p[nt // 2]
                    ib = pinv.tile([128, NTW], F32, tag="pinv")
                    for c in range(4):
                        g = nt * 4 + c
                        nc.tensor.matmul(
                            ib[:, c * 128 : (c + 1) * 128],
                            lhsT=sel_sb[:, g * 128 : (g + 1) * 128],
                            rhs=r16[:],
                            start=True, stop=True,
                        )
                    ev = cfg["evac_eng"][nt]
                    if ev == "D":
                        # no evac: t' STT reads ib from PSUM at fp32 rate
                        for m in range(2):
                            nc.vector.scalar_tensor_tensor(
                                out=t_new[:, m, sl], in0=Y[:, m, sl],
                                scalar=0.0, in1=ib[:],
                                op0=OP.bypass, op1=OP.mult,
                                accum_out=(None if pc is None else pc[:, m, nt : nt + 1]),
                            )
                        return
                    i16 = epool.tile([128, 1, NTW], F16, tag="i16")
                    if ev == "a":
                        nc.scalar.activation(
                            out=i16[:, 0], in_=ib[:], func=ACTF.Copy,
                            bias=0.0, scale=1.0,
                        )
                    else:  # 'p'
                        nc.gpsimd.tensor_scalar(
                            out=i16[:, 0], in0=ib[:], scalar1=0.0,
                            scalar2=None, op0=OP.add,
                        )
                    for m in range(2):
                        nc.vector.scalar_tensor_tensor(
                            out=t_new[:, m, sl], in0=Y[:, m, sl],
                            scalar=0.0, in1=i16[:, 0],
                            op0=OP.bypass, op1=OP.mult,
                            accum_out=(None if pc is None else pc[:, m, nt : nt + 1]),
                        )

                if cfg["rsqrt_pairs"]:
                    stats_chunk(0)
                    stats_chunk(1)
                    rsqrt_pair(0)
                    stats_chunk(2)
                    stats_chunk(3)
                    bcast_tp(0)
                    bcast_tp(1)
                    rsqrt_pair(1)
                    bcast_tp(2)
                    bcast_tp(3)
                else:
                    for nt in range(NT):
                        stats_chunk(nt)
                    rsqrt_pair(0)
                    rsqrt_pair(1)
                    for nt in range(NT):
                        bcast_tp(nt)

                return t_new

            def emit_head(e):
                pc = pooled[e]  # [128, 2, NT] f32 chunk partial sums
                pr = pacc.tile([128, 2], F32, tag=f"pr{e}")
                nc.vector.tensor_tensor(
                    out=pr[:], in0=pc[:, :, 0], in1=pc[:, :, 1], op=OP.add
                )
                p23 = pacc.tile([128, 2], F32, tag=f"p23{e}")
                nc.vector.tensor_tensor(
                    out=p23[:], in0=pc[:, :, 2], in1=pc[:, :, 3], op=OP.add
                )
                nc.vector.tensor_tensor(
                    out=pr[:], in0=pr[:], in1=p23[:], op=OP.add
                )
                pl = pstat.tile([NUM_CLASSES, 1], F32, tag="pstat")
                for k in range(2):
                    nc.tensor.matmul(
                        pl[:],
                        lhsT=hW_sb[:, k, e],
                        rhs=pr[:, k : k + 1],
                        start=(k == 0),
                        stop=(k == 1),
                    )
                nc.vector.tensor_scalar(
                    out=logits_sb[:, e : e + 1], in0=pl[:],
                    scalar1=hb_sb[:, e : e + 1], scalar2=None, op0=OP.add,
                )

            # layer-0 stream t0 = h0 arrives pre-computed via the first DMA

            # ---------------- layers ----------------
            for i in range(N_LAYERS):
                # conv path: S_inf from t[:, :, :JW], tail correction
                v = vpool.tile([128, 2, SEQ], F16, tag="v")
                sinf = small.tile([128, 2], F32, tag="sinf")
                sscr = small.tile([128, 2, JW], F16, tag="sscr")
                conv = small.tile([128, 2, TAIL], F16, tag="conv")
                ut = small.tile([128, 2, TAIL], F16, tag="ut")
                for m in range(2):
                    nc.vector.scalar_tensor_tensor(
                        out=sscr[:, m], in0=t[:, m, 0:JW], scalar=0.0,
                        in1=W1_sb[:, i, m], op0=OP.bypass, op1=OP.mult,
                        accum_out=sinf[:, m : m + 1],
                    )
                    nc.vector.tensor_scalar(
                        out=conv[:, m], in0=Wt_sb[:, i, m],
                        scalar1=sinf[:, m : m + 1], scalar2=None,
                        op0=OP.mult,
                    )
                    # main gelu over [0, SEQ-TAIL), tail handled separately
                    for g0, g1 in cfg.get("gelu_pieces", ((0, 1024), (1024, SEQ - TAIL))):
                        nc.scalar.activation(
                            out=v[:, m, g0:g1], in_=t[:, m, g0:g1],
                            func=GELU, bias=0.0,
                            scale=Dp_sb[:, i, m : m + 1],
                        )
                for m in range(2):
                    nc.vector.scalar_tensor_tensor(
                        out=ut[:, m], in0=t[:, m, SEQ - TAIL :],
                        scalar=Dp_sb[:, i, m : m + 1], in1=conv[:, m],
                        op0=OP.mult, op1=OP.add,
                    )
                    nc.scalar.activation(
                        out=v[:, m, SEQ - TAIL :], in_=ut[:, m],
                        func=GELU, bias=0.0, scale=1.0,
                    )

                # proj matmuls + joins (Y = ps + pbt + t) + sq = Y^2
                Y = ypool.tile([128, 2, SEQ], F16, tag="ypool")
                sq = sqpool.tile([128, 2, SEQ], F16, tag="sq")
                for nt in range(NT):
                    sl = slice(nt * NTW, (nt + 1) * NTW)
                    je = cfg["join_eng"][nt]
                    se = cfg["sq_eng"][nt]
                    for m in range(2):
                        ps = pw.tile([128, NTW], F32, tag="pw")
                        for k in range(2):
                            nc.tensor.matmul(
                                ps[:],
                                lhsT=Pt_sb[:, i, k, m * 128 : (m + 1) * 128],
                                rhs=v[:, k, sl],
                                start=(k == 0),
                                stop=(k == 1) and i != 0,
                            )
                        if i == 0:
                            # center layer-0's x-dependent channel mean
                            nc.tensor.matmul(
                                ps[:],
                                lhsT=corrW_row[:, m * 128 : (m + 1) * 128],
                                rhs=x16_row[:, sl],
                                start=False, stop=True,
                            )
                        if je == "a":
                            w16 = epool.tile([128, NTW], F16, tag=f"w16")
                            nc.scalar.activation(
                                out=w16[:], in_=ps[:], func=ACTF.Identity,
                                bias=pbt_sb[:, i, m : m + 1], scale=1.0,
                            )
                            nc.vector.scalar_tensor_tensor(
                                out=Y[:, m, sl], in0=w16[:],
                                scalar=0.0, in1=t[:, m, sl],
                                op0=OP.bypass, op1=OP.add,
                            )
                        elif je == "p":
                            nc.gpsimd.scalar_tensor_tensor(
                                out=Y[:, m, sl], in0=ps[:],
                                scalar=pbt_sb[:, i, m : m + 1], in1=t[:, m, sl],
                                op0=OP.add, op1=OP.add,
                            )
                        else:  # 'd'
                            nc.vector.scalar_tensor_tensor(
                                out=Y[:, m, sl], in0=ps[:],
                                scalar=pbt_sb[:, i, m : m + 1], in1=t[:, m, sl],
                                op0=OP.add, op1=OP.add,
                            )
                        if se == "a":
                            nc.scalar.activation(
                                out=sq[:, m, sl], in_=Y[:, m, sl],
                                func=ACTF.Square, bias=0.0, scale=1.0,
                            )
                        elif se == "p":
                            nc.gpsimd.tensor_tensor(
                                out=sq[:, m, sl], in0=Y[:, m, sl],
                                in1=Y[:, m, sl], op=OP.mult,
                            )
                        else:  # 'd'
                            nc.vector.scalar_tensor_tensor(
                                out=sq[:, m, sl], in0=Y[:, m, sl],
                                scalar=0.0, in1=Y[:, m, sl],
                                op0=OP.bypass, op1=OP.mult,
                            )

                exit_idx = EXIT_LAYERS.index(i) if i in EXIT_LAYERS else None
                t = tail(Y, sq, i, exit_idx)

            # ---------------- epilogue: pooled reduce + heads ----------------
            for e in range(4):
                emit_head(e)
            nc.sync.dma_start(out=d_out.ap(), in_=logits_sb[:])

    if split:
        _split_drain_waits(nc, mybir)
    return nc


def _forward_fallback(inputs):
    """Numpy-only exact reference computation (general-inputs path)."""
    import math

    erf = np.vectorize(math.erf)
    x = inputs["x"].astype(np.float32)
    h = x[:, :, 0:1] * inputs["in_W"][None, None, :, 0] + inputs["in_b"]
    logits = []
    head = 0
    Lf = np.arange(SEQ, dtype=np.float32)
    for i in range(N_LAYERS):
        A = 1.0 / (1.0 + np.exp(-inputs["A_params"][i].astype(np.float32)))
        K = (
            inputs["C_params"][i][:, None]
            * (A[:, None] ** Lf[None, :])
            * inputs["B_params"][i][:, None]
        ).astype(np.float32)
        ht = np.swapaxes(h, 1, 2).astype(np.float32)
        out = np.empty_like(ht)
        for b in range(x.shape[0]):
            for d in range(D_MODEL):
                c = np.correlate(
                    np.concatenate([np.zeros(SEQ - 1, np.float32), ht[b, d]]),
                    K[d][::-1],
                    mode="valid",
                )
                out[b, d] = c[:SEQ]
        out = out + inputs["D_params"][i][None, :, None] * ht
        u = np.swapaxes(out, 1, 2)
        vg = u * 0.5 * (1.0 + erf(u / np.sqrt(2.0)))
        w = vg.astype(np.float32) @ inputs["proj_W"][i].T + inputs["proj_b"][i]
        y = h + w
        mu = y.mean(-1, keepdims=True)
        var = y.var(-1, keepdims=True)
        h = (y - mu) / np.sqrt(var + LN_EPS) * inputs["ln_g"][i] + inputs["ln_b"][i]
        if i in EXIT_LAYERS:
            pooled = h.mean(axis=1)
            logits.append(pooled @ inputs["head_W"][head].T + inputs["head_b"][head])
            head += 1
    return np.stack(logits, 0).astype(np.float32)


def _run_device(inputs, trace=False):
    from concourse import bass_utils

    key = "nc"
    if key not in _CACHE:
        _CACHE[key] = _build_nc(sim_safe=False)
    nc = _CACHE[key]

    weights = _host_prep(inputs)
    x = np.asarray(inputs["x"], dtype=np.float32)
    inWf = np.asarray(inputs["in_W"], dtype=np.float64)[:, 0]
    inbf = np.asarray(inputs["in_b"], dtype=np.float64)
    in_maps = []
    for b in range(BATCH):
        m = dict(weights)
        xb = x[b, :, 0].astype(np.float64)
        h0 = xb[:, None] * inWf[None, :] + inbf          # [SEQ, 256]
        m["h0_in"] = np.ascontiguousarray(
            h0.reshape(SEQ, 2, 128).transpose(2, 1, 0)
        ).astype(np.float16)                              # [128, 2, SEQ]
        m["x_row16"] = x[b, :, 0].reshape(1, SEQ).astype(np.float16)
        in_maps.append(m)
    res = bass_utils.run_bass_kernel_spmd(
        nc, in_maps, core_ids=list(range(BATCH)), trace=trace
    )
    out = np.empty((4, BATCH, NUM_CLASSES), dtype=np.float32)
    for b in range(BATCH):
        lg = res.results[b]["logits_out"]
        out[:, b, :] = lg.T
    return out, res


def kernel(**inputs):
    inputs = {k: np.asarray(v) for k, v in inputs.items()}
    maxA = float(1.0 / (1.0 + np.exp(-np.abs(inputs["A_params"]).max())))
    fast = (
        np.all(inputs["ln_g"] == 1.0)
        and np.all(inputs["ln_b"] == 0.0)
        and maxA**TAIL < 1e-30
        and inputs["x"].shape == (BATCH, SEQ, 1)
    )
    if not fast:
        return _forward_fallback(inputs)
    out, _ = _run_device(inputs, trace=False)
    return out


# revision 42
# speedup vs baseline: 1.2214x; 1.2214x over previous
"""DeepSSM Trainium2 kernel v3 (8 NeuronCores, data-parallel over batch).

Same math as v2 (conv collapsed to S_inf + tail correction, centered pre-LN
stream) but rebalanced across all four compute engines against the cost
model:

- The residual identity matmul is gone: the PSUM join is now a single
  DVE/Pool scalar_tensor_tensor  Y = (ps + pbt) + t  (PE -1.7us/layer).
- rsqrt is ACT Identity(S+eps) -> DVE reciprocal_approx_fast -> ACT Sqrt
  (3 small ops, ~0.9us/layer cheaper than the 6-op DVE bit-trick).
- sq and t' use TensorScalarPtr (STT with bypass) which the DVE runs in
  4x mode on all-SBUF fp16 operands (193ns vs 327ns per [128,512]).
- Per-chunk engine assignment (join/sq/evac/t') is a CONFIG sweep: part
  of the join+evac load rides the otherwise-idle Pool (gpsimd) engine.
- Exit-layer pooled sums ride the t' STT's accum_out for free.
"""

import numpy as np

D_MODEL = 256
N_LAYERS = 8
NUM_CLASSES = 3
BATCH = 8
SEQ = 2048
JW = 256
TAIL = 256
LN_EPS = 1e-5
EXIT_LAYERS = (1, 3, 5, 7)
NT = 4
NTW = SEQ // NT  # 512
NSUB = 16  # 128-position sub-chunks

_CACHE = {}

# per-nt engine assignment (sweepable):
#   join_eng: 'd' DVE STT (ps+pbt)+t from PSUM; 'p' same on Pool;
#             'a' ACT Identity evac (ps+pbt) then DVE TT add t (2x)
#   sq_eng:   'd' DVE TT Y*Y (2x); 'a' ACT Square(Y); 'p' Pool TT
#   evac_eng: 'a' ACT Copy ib->i16; 'p' Pool TS; 'D' no evac, t' STT reads PSUM
#   rconv_eng: fp32->fp16 convert of the rsqrt result, 'a' ACT / 'd' DVE
CONFIG = dict(
    join_eng=("d", "d", "a", "a"),
    sq_eng=("d", "p", "p", "a"),
    sq_merge=True,
    evac_eng=("a", "a", "a", "a"),
    tp_eng=("d", "d", "p", "d"),
    rconv_eng=("a", "a"),
    psum_recip=True,
    gelu_pieces=((0, 512), (512, 1024), (1024, 1536), (1536, 1792)),
    gelu_interleave=True,
    rsqrt_pairs=True,
    pw_bufs=3,
    h0_late=False,
    defer=True,
)


def _host_prep(inputs):
    f64 = np.float64
    A = 1.0 / (1.0 + np.exp(-inputs["A_params"].astype(f64)))  # [nl, d]
    lnA = np.log(A)
    CB = inputs["C_params"].astype(f64) * inputs["B_params"].astype(f64)
    j1 = np.arange(JW, dtype=f64)
    lt = (TAIL - 1.0) - np.arange(TAIL, dtype=f64)
    W1 = np.exp(lnA[:, :, None] * j1[None, None, :])            # [nl, d, JW]
    Wt = CB[:, :, None] * np.exp(lnA[:, :, None] * lt[None, None, :])

    def to_chunks(T, dt):  # [nl, d, l] -> [128, nl, 2, l]
        return np.ascontiguousarray(
            T.reshape(N_LAYERS, 2, 128, -1).transpose(2, 0, 1, 3)
        ).astype(dt)

    pW = inputs["proj_W"].astype(f64)                            # [nl, do, di]
    pWc = pW - pW.mean(axis=1, keepdims=True)
    PtT_all = np.ascontiguousarray(
        pWc.transpose(0, 2, 1).reshape(N_LAYERS, 2, 128, D_MODEL).transpose(2, 0, 1, 3)
    ).astype(np.float16)                                          # [128,nl,2,256]

    Dp_all = np.ascontiguousarray(
        inputs["D_params"].reshape(N_LAYERS, 2, 128).transpose(2, 0, 1)
    ).astype(np.float32)
    pb = inputs["proj_b"].astype(f64)
    pbt = pb - pb.mean(axis=1, keepdims=True)
    pbt_all = np.ascontiguousarray(
        pbt.reshape(N_LAYERS, 2, 128).transpose(2, 0, 1)
    ).astype(np.float32)

    # layer-0 stream is RAW h0 = inW*x + in_b; its centering for the LN
    # stats rides in via corrW (x-dependent) and the pbt[0] constant.
    inW = inputs["in_W"][:, 0].astype(f64)
    inb = inputs["in_b"].astype(f64)
    corrW_row = np.full((1, D_MODEL), -inW.mean(), dtype=np.float16)
    pbt_all[:, 0, :] -= np.float32(inb.mean())

    hW = inputs["head_W"].astype(f64) / SEQ                      # [4, nc, d]
    headWT_all = np.ascontiguousarray(
        hW.transpose(2, 0, 1).reshape(2, 128, 4, NUM_CLASSES).transpose(1, 0, 2, 3)
    ).astype(np.float32)                                          # [128,2,4,3]
    headb_all = np.ascontiguousarray(
        inputs["head_b"].astype(np.float32).T.reshape(NUM_CLASSES, 4)
    )

    sel = np.zeros((8, NSUB * 128), np.float16)
    for g in range(NSUB):
        sel[g % 8, g * 128:(g + 1) * 128] = 1.0

    return dict(
        W1_all=to_chunks(W1, np.float16),
        Wt_all=to_chunks(Wt, np.float16),
        PtT_all=PtT_all,
        Dp_all=Dp_all,
        pbt_all=pbt_all,
        corrW_row=corrW_row,
        sel_all=sel,
        ident_in=np.ascontiguousarray(np.eye(128, dtype=np.float16)),
        onesI_in=np.ascontiguousarray(
            np.tile(np.eye(8, dtype=np.float16)[None] / D_MODEL, (128, 1, 1))
        ),
        headWT_all=headWT_all,
        headb_all=headb_all,
    )


def _split_drain_waits(nc, mybir, maxw=1):
    """Hoist excess sync waits onto same-engine NOPs (walrus ISA limit)."""
    for f in nc.m.functions:
        for blk in f.blocks:
            insts = list(blk.instructions)
            changed = False
            new_list = []
            for ins in insts:
                w = (
                    list(ins.sync_info.on_wait)
                    if ins.sync_info and ins.sync_info.on_wait
                    else []
                )
                if len(w) > maxw:
                    changed = True
                    extra, keep = w[:-maxw], w[-maxw:]
                    for j in range(0, len(extra), maxw):
                        nop = mybir.InstNoOp(
                            name=f"{ins.name}-wsplit{j}", ins=[], outs=[]
                        )
                        nop.engine = ins.engine
                        nop.sync_info = mybir.SyncInfo(
                            on_wait=extra[j : j + maxw], on_update=[]
                        )
                        new_list.append(nop)
                    ins.sync_info.on_wait = keep
                new_list.append(ins)
            if changed:
                blk.instructions = new_list


def _build_nc(sim_safe=False, split=True, cfg=None):
    import concourse.bass as bass
    import concourse.tile as tile
    import concourse.mybir as mybir

    F32 = mybir.dt.float32
    F16 = mybir.dt.float16
    OP = mybir.AluOpType
    ACTF = mybir.ActivationFunctionType
    GELU = ACTF.Sigmoid if sim_safe else ACTF.Gelu
    if cfg is None:
        cfg = CONFIG

    nc = bass.Bass("TRN2", target_bir_lowering=False, debug=False)

    d_h0 = nc.dram_tensor("h0_in", [128, 2, SEQ], F16, kind="ExternalInput")
    d_W1 = nc.dram_tensor("W1_all", [128, N_LAYERS, 2, JW], F16, kind="ExternalInput")
    d_Wt = nc.dram_tensor("Wt_all", [128, N_LAYERS, 2, TAIL], F16, kind="ExternalInput")
    d_Pt = nc.dram_tensor("PtT_all", [128, N_LAYERS, 2, D_MODEL], F16, kind="ExternalInput")
    d_Dp = nc.dram_tensor("Dp_all", [128, N_LAYERS, 2], F32, kind="ExternalInput")
    d_pbt = nc.dram_tensor("pbt_all", [128, N_LAYERS, 2], F32, kind="ExternalInput")
    d_corrW = nc.dram_tensor("corrW_row", [1, D_MODEL], F16, kind="ExternalInput")
    d_x16 = nc.dram_tensor("x_row16", [1, SEQ], F16, kind="ExternalInput")
    d_ident = nc.dram_tensor("ident_in", [128, 128], F16, kind="ExternalInput")
    d_sel = nc.dram_tensor("sel_all", [8, NSUB * 128], F16, kind="ExternalInput")
    d_onesI = nc.dram_tensor("onesI_in", [128, 8, 8], F16, kind="ExternalInput")
    d_hW = nc.dram_tensor("headWT_all", [128, 2, 4, NUM_CLASSES], F32, kind="ExternalInput")
    d_hb = nc.dram_tensor("headb_all", [NUM_CLASSES, 4], F32, kind="ExternalInput")
    d_out = nc.dram_tensor("logits_out", [NUM_CLASSES, 4], F32, kind="ExternalOutput")

    with tile.TileContext(nc) as tc:
        from contextlib import ExitStack

        ctx = ExitStack()
        with ctx:
            const = ctx.enter_context(tc.tile_pool(name="const", bufs=1))
            stream = ctx.enter_context(tc.tile_pool(name="stream", bufs=cfg.get("stream_bufs", 3)))
            ypool = ctx.enter_context(tc.tile_pool(name="ypool", bufs=cfg.get("y_bufs", 2)))
            vpool = ctx.enter_context(tc.tile_pool(name="vpool", bufs=cfg.get("v_bufs", 2)))
            sqpool = ctx.enter_context(tc.tile_pool(name="sqpool", bufs=cfg.get("sq_bufs", 2)))
            epool = ctx.enter_context(tc.tile_pool(name="epool", bufs=cfg.get("e_bufs", 2)))
            small = ctx.enter_context(tc.tile_pool(name="small", bufs=cfg.get("small_bufs", 2)))
            stat = ctx.enter_context(tc.tile_pool(name="stat", bufs=cfg.get("stat_bufs", 2)))
            pacc = ctx.enter_context(tc.tile_pool(name="pacc", bufs=5))
            pw = ctx.enter_context(tc.tile_pool(name="pw", bufs=cfg.get("pw_bufs", 4), space="PSUM"))
            pstat = ctx.enter_context(tc.tile_pool(name="pstat", bufs=1, space="PSUM"))
            pinv = ctx.enter_context(tc.tile_pool(name="pinv", bufs=2, space="PSUM"))

            # ---- constants / weights to SBUF ----
            # SP issues DMAs in-order: layer-0's working set first, cold
            # layers and head tables last.
            t = stream.tile([128, 2, SEQ], F16, tag="stream")
            nc.sync.dma_start(out=t[:, :, 0:NTW], in_=d_h0.ap()[:, :, 0:NTW])
            Dp_sb = const.tile([128, N_LAYERS, 2], F32)
            nc.sync.dma_start(out=Dp_sb[:], in_=d_Dp.ap())
            Pt_sb = const.tile([128, N_LAYERS, 2, D_MODEL], F16)
            W1_sb = const.tile([128, N_LAYERS, 2, JW], F16)
            Wt_sb = const.tile([128, N_LAYERS, 2, TAIL], F16)
            nc.sync.dma_start(out=W1_sb[:, 0], in_=d_W1.ap()[:, 0])
            nc.sync.dma_start(out=t[:, :, NTW : 2 * NTW], in_=d_h0.ap()[:, :, NTW : 2 * NTW])
            nc.sync.dma_start(out=Wt_sb[:, 0], in_=d_Wt.ap()[:, 0])
            if cfg.get("h0_late"):
                nc.sync.dma_start(out=t[:, :, 2 * NTW : 3 * NTW], in_=d_h0.ap()[:, :, 2 * NTW : 3 * NTW])
                nc.sync.dma_start(out=Pt_sb[:, 0], in_=d_Pt.ap()[:, 0])
                nc.sync.dma_start(out=t[:, :, 3 * NTW :], in_=d_h0.ap()[:, :, 3 * NTW :])
            else:
                nc.sync.dma_start(out=Pt_sb[:, 0], in_=d_Pt.ap()[:, 0])
                nc.sync.dma_start(out=t[:, :, 2 * NTW : 3 * NTW], in_=d_h0.ap()[:, :, 2 * NTW : 3 * NTW])
                nc.sync.dma_start(out=t[:, :, 3 * NTW :], in_=d_h0.ap()[:, :, 3 * NTW :])
            pbt_sb = const.tile([128, N_LAYERS, 2], F32)
            nc.sync.dma_start(out=pbt_sb[:], in_=d_pbt.ap())
            x16_row = const.tile([1, SEQ], F16)
            nc.sync.dma_start(out=x16_row[:], in_=d_x16.ap())
            corrW_row = const.tile([1, D_MODEL], F16)
            nc.sync.dma_start(out=corrW_row[:], in_=d_corrW.ap())
            onesI_sb = const.tile([128, 8, 8], F16)
            nc.sync.dma_start(out=onesI_sb[:], in_=d_onesI.ap())
            ident_sb = const.tile([128, 128], F16)
            nc.sync.dma_start(out=ident_sb[:], in_=d_ident.ap())
            sel_sb = const.tile([8, NSUB * 128], F16)
            nc.sync.dma_start(out=sel_sb[:], in_=d_sel.ap())
            for i in range(1, N_LAYERS):
                nc.sync.dma_start(out=W1_sb[:, i], in_=d_W1.ap()[:, i])
                nc.sync.dma_start(out=Wt_sb[:, i], in_=d_Wt.ap()[:, i])
                nc.sync.dma_start(out=Pt_sb[:, i], in_=d_Pt.ap()[:, i])
            hW_sb = const.tile([128, 2, 4, NUM_CLASSES], F32)
            nc.sync.dma_start(out=hW_sb[:], in_=d_hW.ap())
            hb_sb = const.tile([NUM_CLASSES, 4], F32)
            nc.sync.dma_start(out=hb_sb[:], in_=d_hb.ap())
            eps_col = const.tile([8, 1], F32)
            nc.vector.memset(eps_col, LN_EPS)
            # eps injection for the stats PSUM accumulation: a [1,8] x [1,128]
            # rank-1 matmul adding eps to every stat cell.  1e-5 is subnormal
            # in fp16, so split it 2.56e-3 * 3.90625e-3.
            epsT_row = const.tile([1, 8], F16)
            nc.vector.memset(epsT_row, LN_EPS * 256.0)
            ones_row = const.tile([1, 128], F16)
            nc.vector.memset(ones_row, 1.0 / 256.0)

            logits_sb = const.tile([NUM_CLASSES, 4], F32)
            pooled = {}

            # ---------------- shared tail: Y -> t_next ----------------
            def tail(Y, sq, li, exit_idx, ps_tiles):
                """Y fp16 [128,2,SEQ] (centered pre-LN) -> t fp16 (normalized).

                stats (tiny PE matmuls into per-pair PSUM stat tiles) ->
                rsqrt (ACT eps-evac, DVE reciprocal_approx_fast, ACT Sqrt)
                -> PE selector-broadcast -> evac -> t' STT (4x, accum_out
                doubles as exit-layer pooled partial sums)."""
                S_psA = pstat.tile([8, 128], F32, tag="pstat")
                S_psB = pstat.tile([8, 128], F32, tag="pstatB")
                S_psp = {0: S_psA, 1: S_psB}
                t_new = stream.tile([128, 2, SEQ], F16, tag="stream")
                r16p = {}

                PR = cfg.get("psum_recip")

                def sq_chunk(nt):
                    # emitted HERE (not in the proj loop) so pair-0's recip
                    # isn't queued behind pair-1's sq in DVE program order
                    sl = slice(nt * NTW, (nt + 1) * NTW)
                    se = cfg["sq_eng"][nt]
                    if se == "P":
                        # ident chunks: ps already holds ps+t, so Square can
                        # read PSUM with the pbt bias — no join dependency
                        for m in range(2):
                            nc.scalar.activation(
                                out=sq[:, m, sl], in_=ps_tiles[nt, m][:],
                                func=ACTF.Square,
                                bias=pbt_sb[:, li, m : m + 1], scale=1.0,
                            )
                        return
                    if se == "d" and cfg.get("sq_merge"):
                        nc.vector.tensor_tensor(
                            out=sq[:, :, sl], in0=Y[:, :, sl],
                            in1=Y[:, :, sl], op=OP.mult,
                        )
                        return
                    for m in range(2):
                        if se == "a":
                            nc.scalar.activation(
                                out=sq[:, m, sl], in_=Y[:, m, sl],
                                func=ACTF.Square, bias=0.0, scale=1.0,
                            )
                        elif se == "p":
                            nc.gpsimd.tensor_tensor(
                                out=sq[:, m, sl], in0=Y[:, m, sl],
                                in1=Y[:, m, sl], op=OP.mult,
                            )
                        else:
                            nc.vector.tensor_tensor(
                                out=sq[:, m, sl], in0=Y[:, m, sl],
                                in1=Y[:, m, sl], op=OP.mult,
                            )

                def stats_chunk(nt):
                    sq_chunk(nt)
                    if PR and nt % 2 == 0:
                        # open the pair's accumulation group with the eps
                        # rank-1 term so the DVE reciprocal can read PSUM raw
                        nc.tensor.matmul(
                            S_psp[nt // 2][:],
                            lhsT=epsT_row[:],
                            rhs=ones_row[:],
                            start=True, stop=False,
                        )
                    for c in range(4):
                        g = nt * 4 + c
                        csl = slice(g * 128, (g + 1) * 128)
                        for m in range(2):
                            nc.tensor.matmul(
                                S_psp[g // 8][:],
                                lhsT=onesI_sb[:, g % 8, :],
                                rhs=sq[:, m, csl],
                                start=(not PR) and (g % 8 == 0 and m == 0),
                                stop=(g % 8 == 7 and m == 1),
                            )

                def rsqrt_pair(p):
                    # inv = 1/sqrt(S+eps).  psum_recip: eps was accumulated by
                    # PE, so DVE reciprocal_approx_fast (custom op, ~18 bits)
                    # reads the PSUM stat tile directly and ACT Sqrt finishes
                    # straight to fp16 — two ops, no separate eps/convert.
                    y32 = stat.tile([8, 128], F32, tag="y32")
                    r16 = stat.tile([8, 128], F16, tag="r16")
                    r16p[p] = r16
                    if PR:
                        nc.vector.reciprocal(out=y32[:], in_=S_psp[p][:])
                        nc.scalar.activation(
                            out=r16[:], in_=y32[:], func=ACTF.Sqrt,
                            bias=0.0, scale=1.0,
                        )
                        return
                    v16 = stat.tile([8, 128], F32, tag="v16")
                    nc.scalar.activation(
                        out=v16[:], in_=S_psp[p][:], func=ACTF.Sqrt,
                        bias=eps_col[:], scale=1.0,
                    )
                    nc.vector.reciprocal(out=y32[:], in_=v16[:])
                    if cfg.get("rconv_eng", ("a", "a"))[p] == "a":
                        nc.scalar.activation(
                            out=r16[:], in_=y32[:], func=ACTF.Copy,
                            bias=0.0, scale=1.0,
                        )
                    else:
                        nc.vector.tensor_scalar(
                            out=r16[:], in0=y32[:], scalar1=0.0,
                            scalar2=None, op0=OP.add,
                        )

                def bcast_tp(nt):
                    # reads only this nt-pair's r16 tile so pair-0 broadcasts
                    # don't wait on pair-1's rsqrt
                    sl = slice(nt * NTW, (nt + 1) * NTW)
                    r16 = r16p[nt // 2]
                    ib = pinv.tile([128, NTW], F32, tag="pinv")
                    for c in range(4):
                        g = nt * 4 + c
                        nc.tensor.matmul(
                            ib[:, c * 128 : (c + 1) * 128],
                            lhsT=sel_sb[:, g * 128 : (g + 1) * 128],
                            rhs=r16[:],
                            start=True, stop=True,
                        )
                    ev = cfg["evac_eng"][nt]
                    if ev == "D":
                        # no evac: t' STT reads ib from PSUM at fp32 rate
                        for m in range(2):
                            nc.vector.scalar_tensor_tensor(
                                out=t_new[:, m, sl], in0=Y[:, m, sl],
                                scalar=0.0, in1=ib[:],
                                op0=OP.bypass, op1=OP.mult,
                            )
                        return
                    i16 = epool.tile([128, 1, NTW], F16, tag="i16")
                    if ev == "a":
                        nc.scalar.activation(
                            out=i16[:, 0], in_=ib[:], func=ACTF.Copy,
                            bias=0.0, scale=1.0,
                        )
                    elif ev == "d":
                        nc.vector.tensor_scalar(
                            out=i16[:, 0], in0=ib[:], scalar1=0.0,
                            scalar2=None, op0=OP.add,
                        )
                    else:  # 'p'
                        nc.gpsimd.tensor_scalar(
                            out=i16[:, 0], in0=ib[:], scalar1=0.0,
                            scalar2=None, op0=OP.add,
                        )
                    tp = cfg.get("tp_eng", ("d",) * 4)[nt]
                    for m in range(2):
                        if tp == "p":
                            nc.gpsimd.tensor_tensor(
                                out=t_new[:, m, sl], in0=Y[:, m, sl],
                                in1=i16[:, 0], op=OP.mult,
                            )
                        else:
                            nc.vector.tensor_tensor(
                                out=t_new[:, m, sl], in0=Y[:, m, sl],
                                in1=i16[:, 0], op=OP.mult,
                            )

                # part A: everything pair-0 needs, plus pair-1's stats.
                if cfg.get("tp_first"):
                    stats_chunk(0)
                    stats_chunk(1)
                    rsqrt_pair(0)
                    bcast_tp(0)
                    bcast_tp(1)
                    stats_chunk(2)
                    stats_chunk(3)
                else:
                    stats_chunk(0)
                    stats_chunk(1)
                    rsqrt_pair(0)
                    stats_chunk(2)
                    stats_chunk(3)
                    bcast_tp(0)
                    bcast_tp(1)

                def finish():
                    # pair-1 tail — deferrable past the next layer's head so
                    # its ops don't block gelu/proj in engine program order.
                    rsqrt_pair(1)
                    bcast_tp(2)
                    bcast_tp(3)
                    if exit_idx is not None:
                        # pooled = sum_l t_new via 4x-mode TS accum; the dead
                        # sq tile is the throwaway elementwise output.
                        pe_ = cfg.get("pooled_eng", "d")
                        peng = nc.vector if pe_ == "d" else nc.gpsimd
                        nacc = cfg.get("pooled_nacc", 2)
                        w = SEQ // nacc
                        pc = pacc.tile([128, 2, nacc], F32, tag="pacc")
                        pooled[exit_idx] = pc
                        for m in range(2):
                            for h in range(nacc):
                                hsl = slice(h * w, (h + 1) * w)
                                peng.tensor_scalar(
                                    out=sq[:, m, hsl], in0=t_new[:, m, hsl],
                                    scalar1=0.0, scalar2=0.0, op0=OP.add,
                                    op1=OP.add,
                                    accum_out=pc[:, m, h : h + 1],
                                )

                return t_new, finish

            def emit_head(e):
                pc = pooled[e]  # [128, 2, nacc] f32 chunk partial sums
                pr = pacc.tile([128, 2], F32, tag=f"pr{e}")
                nc.vector.tensor_tensor(
                    out=pr[:], in0=pc[:, :, 0], in1=pc[:, :, 1], op=OP.add
                )
                if pc.shape[2] == 4:
                    p23 = pacc.tile([128, 2], F32, tag=f"p23{e}")
                    nc.vector.tensor_tensor(
                        out=p23[:], in0=pc[:, :, 2], in1=pc[:, :, 3], op=OP.add
                    )
                    nc.vector.tensor_tensor(
                        out=pr[:], in0=pr[:], in1=p23[:], op=OP.add
                    )
                pl = pstat.tile([NUM_CLASSES, 1], F32, tag="pstat")
                for k in range(2):
                    nc.tensor.matmul(
                        pl[:],
                        lhsT=hW_sb[:, k, e],
                        rhs=pr[:, k : k + 1],
                        start=(k == 0),
                        stop=(k == 1),
                    )
                nc.vector.tensor_scalar(
                    out=logits_sb[:, e : e + 1], in0=pl[:],
                    scalar1=hb_sb[:, e : e + 1], scalar2=None, op0=OP.add,
                )

            # layer-0 stream t0 = h0 arrives pre-computed via the first DMA

            # ---------------- layers ----------------
            # Emission is interleaved across layers: layer i-1's pair-1 tail
            # (finish_prev) is emitted AFTER layer i's first gelu piece and
            # nt=0 proj/join, so the new layer's head isn't queued behind the
            # old layer's tail in ACT/PE/DVE program order.
            finish_prev = None
            for i in range(N_LAYERS):
                v = vpool.tile([128, 2, SEQ], F16, tag="v")
                sinf = small.tile([128, 2], F32, tag="sinf")
                sscr = small.tile([128, 2, JW], F16, tag="sscr")
                conv = small.tile([128, 2, TAIL], F16, tag="conv")
                ut = small.tile([128, 2, TAIL], F16, tag="ut")
                conv_eng = nc.gpsimd if cfg.get("conv_pool") else nc.vector
                for m in range(2):
                    conv_eng.scalar_tensor_tensor(
                        out=sscr[:, m], in0=t[:, m, 0:JW], scalar=0.0,
                        in1=W1_sb[:, i, m], op0=OP.bypass, op1=OP.mult,
                        accum_out=sinf[:, m : m + 1],
                    )
                    conv_eng.tensor_scalar(
                        out=conv[:, m], in0=Wt_sb[:, i, m],
                        scalar1=sinf[:, m : m + 1], scalar2=None,
                        op0=OP.mult,
                    )

                pieces = cfg.get("gelu_pieces")
                assert len(pieces) >= 3 and pieces[0][1] >= NTW and pieces[1][1] >= 2 * NTW

                def gelu_piece(k, i=i, v=v):
                    g0, g1 = pieces[k]
                    for m in range(2):
                        nc.scalar.activation(
                            out=v[:, m, g0:g1], in_=t[:, m, g0:g1],
                            func=GELU, bias=0.0,
                            scale=Dp_sb[:, i, m : m + 1],
                        )

                Y = ypool.tile([128, 2, SEQ], F16, tag="ypool")
                sq = sqpool.tile([128, 2, SEQ], F16, tag="sq")
                ps_tiles = {}

                def proj_join(nt, i=i, v=v, Y=Y, t=t, ps_tiles=ps_tiles):
                    sl = slice(nt * NTW, (nt + 1) * NTW)
                    je = cfg["join_eng"][nt]
                    ident = cfg.get("ident_nt", (0, 0, 0, 0))[nt]
                    for m in range(2):
                        ps = pw.tile([128, NTW], F32, tag="pw")
                        ps_tiles[nt, m] = ps
                        if ident:
                            nc.tensor.matmul(
                                ps[:], lhsT=ident_sb[:], rhs=t[:, m, sl],
                                start=True, stop=False,
                            )
                        for k in range(2):
                            nc.tensor.matmul(
                                ps[:],
                                lhsT=Pt_sb[:, i, k, m * 128 : (m + 1) * 128],
                                rhs=v[:, k, sl],
                                start=(k == 0) and not ident,
                                stop=(k == 1) and i != 0,
                            )
                        if i == 0:
                            # center layer-0's x-dependent channel mean
                            nc.tensor.matmul(
                                ps[:],
                                lhsT=corrW_row[:, m * 128 : (m + 1) * 128],
                                rhs=x16_row[:, sl],
                                start=False, stop=True,
                            )
                        if ident:
                            # +t already in PSUM: the join is a pure bias evac
                            if je == "a":
                                nc.scalar.activation(
                                    out=Y[:, m, sl], in_=ps[:], func=ACTF.Identity,
                                    bias=pbt_sb[:, i, m : m + 1], scale=1.0,
                                )
                            elif je == "p":
                                nc.gpsimd.tensor_scalar(
                                    out=Y[:, m, sl], in0=ps[:],
                                    scalar1=pbt_sb[:, i, m : m + 1], scalar2=None,
                                    op0=OP.add,
                                )
                            else:
                                nc.vector.tensor_scalar(
                                    out=Y[:, m, sl], in0=ps[:],
                                    scalar1=pbt_sb[:, i, m : m + 1], scalar2=None,
                                    op0=OP.add,
                                )
                        elif je in ("a", "A"):
                            w16 = epool.tile([128, NTW], F16, tag="w16")
                            nc.scalar.activation(
                                out=w16[:], in_=ps[:], func=ACTF.Identity,
                                bias=pbt_sb[:, i, m : m + 1], scale=1.0,
                            )
                            if je == "A":  # SBUF add half on Pool
                                nc.gpsimd.tensor_tensor(
                                    out=Y[:, m, sl], in0=w16[:],
                                    in1=t[:, m, sl], op=OP.add,
                                )
                            else:
                                nc.vector.tensor_tensor(
                                    out=Y[:, m, sl], in0=w16[:],
                                    in1=t[:, m, sl], op=OP.add,
                                )
                        else:  # 'd'
                            nc.vector.scalar_tensor_tensor(
                                out=Y[:, m, sl], in0=ps[:],
                                scalar=pbt_sb[:, i, m : m + 1], in1=t[:, m, sl],
                                op0=OP.add, op1=OP.add,
                            )

                gelu_piece(0)
                proj_join(0)
                if finish_prev is not None and cfg.get("defer", True):
                    finish_prev()
                    finish_prev = None
                gelu_piece(1)
                proj_join(1)
                for k in range(2, len(pieces)):
                    gelu_piece(k)
                for m in range(2):
                    conv_eng.scalar_tensor_tensor(
                        out=ut[:, m], in0=t[:, m, SEQ - TAIL :],
                        scalar=Dp_sb[:, i, m : m + 1], in1=conv[:, m],
                        op0=OP.mult, op1=OP.add,
                    )
                    nc.scalar.activation(
                        out=v[:, m, SEQ - TAIL :], in_=ut[:, m],
                        func=GELU, bias=0.0, scale=1.0,
                    )
                proj_join(2)
                proj_join(3)

                exit_idx = EXIT_LAYERS.index(i) if i in EXIT_LAYERS else None
                t, fin = tail(Y, sq, i, exit_idx, ps_tiles)
                if finish_prev is not None:
                    finish_prev()
                finish_prev = fin

            finish_prev()
            # ---------------- epilogue: pooled reduce + heads ----------------
            for e in range(4):
                emit_head(e)
            nc.sync.dma_start(out=d_out.ap(), in_=logits_sb[:])

    if split:
        _split_drain_waits(nc, mybir)
    return nc


def _forward_fallback(inputs):
    """Numpy-only exact reference computation (general-inputs path)."""
    import math

    erf = np.vectorize(math.erf)
    x = inputs["x"].astype(np.float32)
    h = x[:, :, 0:1] * inputs["in_W"][None, None, :, 0] + inputs["in_b"]
    logits = []
    head = 0
    Lf = np.arange(SEQ, dtype=np.float32)
    for i in range(N_LAYERS):
        A = 1.0 / (1.0 + np.exp(-inputs["A_params"][i].astype(np.float32)))
        K = (
            inputs["C_params"][i][:, None]
            * (A[:, None] ** Lf[None, :])
            * inputs["B_params"][i][:, None]
        ).astype(np.float32)
        ht = np.swapaxes(h, 1, 2).astype(np.float32)
        out = np.empty_like(ht)
        for b in range(x.shape[0]):
            for d in range(D_MODEL):
                c = np.correlate(
                    np.concatenate([np.zeros(SEQ - 1, np.float32), ht[b, d]]),
                    K[d][::-1],
                    mode="valid",
                )
                out[b, d] = c[:SEQ]
        out = out + inputs["D_params"][i][None, :, None] * ht
        u = np.swapaxes(out, 1, 2)
        vg = u * 0.5 * (1.0 + erf(u / np.sqrt(2.0)))
        w = vg.astype(np.float32) @ inputs["proj_W"][i].T + inputs["proj_b"][i]
        y = h + w
        mu = y.mean(-1, keepdims=True)
        var = y.var(-1, keepdims=True)
        h = (y - mu) / np.sqrt(var + LN_EPS) * inputs["ln_g"][i] + inputs["ln_b"][i]
        if i in EXIT_LAYERS:
            pooled = h.mean(axis=1)
            logits.append(pooled @ inputs["head_W"][head].T + inputs["head_b"][head])
            head += 1
    return np.stack(logits, 0).astype(np.float32)


def _run_device(inputs, trace=False):
    from concourse import bass_utils

    key = "nc"
    if key not in _CACHE:
        _CACHE[key] = _build_nc(sim_safe=False)
    nc = _CACHE[key]

    weights = _host_prep(inputs)
    x = np.asarray(inputs["x"], dtype=np.float32)
    inWf = np.asarray(inputs["in_W"], dtype=np.float64)[:, 0]
    inbf = np.asarray(inputs["in_b"], dtype=np.float64)
    in_maps = []
    for b in range(BATCH):
        m = dict(weights)
        xb = x[b, :, 0].astype(np.float64)
        h0 = xb[:, None] * inWf[None, :] + inbf          # [SEQ, 256]
        m["h0_in"] = np.ascontiguousarray(
            h0.reshape(SEQ, 2, 128).transpose(2, 1, 0)
        ).astype(np.float16)                              # [128, 2, SEQ]
        m["x_row16"] = x[b, :, 0].reshape(1, SEQ).astype(np.float16)
        in_maps.append(m)
    res = bass_utils.run_bass_kernel_spmd(
        nc, in_maps, core_ids=list(range(BATCH)), trace=trace
    )
    out = np.empty((4, BATCH, NUM_CLASSES), dtype=np.float32)
    for b in range(BATCH):
        lg = res.results[b]["logits_out"]
        out[:, b, :] = lg.T
    return out, res


def kernel(**inputs):
    inputs = {k: np.asarray(v) for k, v in inputs.items()}
    maxA = float(1.0 / (1.0 + np.exp(-np.abs(inputs["A_params"]).max())))
    fast = (
        np.all(inputs["ln_g"] == 1.0)
        and np.all(inputs["ln_b"] == 0.0)
        and maxA**TAIL < 1e-30
        and inputs["x"].shape == (BATCH, SEQ, 1)
    )
    if not fast:
        return _forward_fallback(inputs)
    out, _ = _run_device(inputs, trace=False)
    return out


# revision 53
# speedup vs baseline: 1.2250x; 1.0029x over previous
"""DeepSSM Trainium2 kernel v3 (8 NeuronCores, data-parallel over batch).

Same math as v2 (conv collapsed to S_inf + tail correction, centered pre-LN
stream) but rebalanced across all four compute engines against the cost
model:

- The residual identity matmul is gone: the PSUM join is now a single
  DVE/Pool scalar_tensor_tensor  Y = (ps + pbt) + t  (PE -1.7us/layer).
- rsqrt is ACT Identity(S+eps) -> DVE reciprocal_approx_fast -> ACT Sqrt
  (3 small ops, ~0.9us/layer cheaper than the 6-op DVE bit-trick).
- sq and t' use TensorScalarPtr (STT with bypass) which the DVE runs in
  4x mode on all-SBUF fp16 operands (193ns vs 327ns per [128,512]).
- Per-chunk engine assignment (join/sq/evac/t') is a CONFIG sweep: part
  of the join+evac load rides the otherwise-idle Pool (gpsimd) engine.
- Exit-layer pooled sums ride the t' STT's accum_out for free.
"""

import numpy as np

D_MODEL = 256
N_LAYERS = 8
NUM_CLASSES = 3
BATCH = 8
SEQ = 2048
JW = 256
TAIL = 256
LN_EPS = 1e-5
EXIT_LAYERS = (1, 3, 5, 7)
NT = 4
NTW = SEQ // NT  # 512
NSUB = 16  # 128-position sub-chunks

_CACHE = {}

# per-nt engine assignment (sweepable):
#   join_eng: 'd' DVE STT (ps+pbt)+t from PSUM; 'p' same on Pool;
#             'a' ACT Identity evac (ps+pbt) then DVE TT add t (2x)
#   sq_eng:   'd' DVE TT Y*Y (2x); 'a' ACT Square(Y); 'p' Pool TT
#   evac_eng: 'a' ACT Copy ib->i16; 'p' Pool TS; 'D' no evac, t' STT reads PSUM
#   rconv_eng: fp32->fp16 convert of the rsqrt result, 'a' ACT / 'd' DVE
CONFIG = dict(
    join_eng=("d", "d", "a", "a"),
    sq_eng=("d", "p", "p", "a"),
    sq_merge=True,
    evac_eng=("a", "a", "a", "a"),
    tp_eng=("d", "d", "p", "d"),
    rconv_eng=("a", "a"),
    psum_recip=True,
    gelu_pieces=((0, 512), (512, 1024), (1024, 1536), (1536, 1792)),
    gelu_interleave=True,
    rsqrt_pairs=True,
    pw_bufs=3,
    h0_late=False,
    defer=True,
)


def _host_prep(inputs):
    f64 = np.float64
    A = 1.0 / (1.0 + np.exp(-inputs["A_params"].astype(f64)))  # [nl, d]
    lnA = np.log(A)
    CB = inputs["C_params"].astype(f64) * inputs["B_params"].astype(f64)
    j1 = np.arange(JW, dtype=f64)
    lt = (TAIL - 1.0) - np.arange(TAIL, dtype=f64)
    W1 = np.exp(lnA[:, :, None] * j1[None, None, :])            # [nl, d, JW]
    Wt = CB[:, :, None] * np.exp(lnA[:, :, None] * lt[None, None, :])

    def to_chunks(T, dt):  # [nl, d, l] -> [128, nl, 2, l]
        return np.ascontiguousarray(
            T.reshape(N_LAYERS, 2, 128, -1).transpose(2, 0, 1, 3)
        ).astype(dt)

    pW = inputs["proj_W"].astype(f64)                            # [nl, do, di]
    pWc = pW - pW.mean(axis=1, keepdims=True)
    PtT_all = np.ascontiguousarray(
        pWc.transpose(0, 2, 1).reshape(N_LAYERS, 2, 128, D_MODEL).transpose(2, 0, 1, 3)
    ).astype(np.float16)                                          # [128,nl,2,256]

    Dp_all = np.ascontiguousarray(
        inputs["D_params"].reshape(N_LAYERS, 2, 128).transpose(2, 0, 1)
    ).astype(np.float32)
    pb = inputs["proj_b"].astype(f64)
    pbt = pb - pb.mean(axis=1, keepdims=True)
    pbt_all = np.ascontiguousarray(
        pbt.reshape(N_LAYERS, 2, 128).transpose(2, 0, 1)
    ).astype(np.float32)

    # layer-0 stream is RAW h0 = inW*x + in_b; its centering for the LN
    # stats rides in via corrW (x-dependent) and the pbt[0] constant.
    inW = inputs["in_W"][:, 0].astype(f64)
    inb = inputs["in_b"].astype(f64)
    corrW_row = np.full((1, D_MODEL), -inW.mean(), dtype=np.float16)
    pbt_all[:, 0, :] -= np.float32(inb.mean())

    hW = inputs["head_W"].astype(f64) / SEQ                      # [4, nc, d]
    headWT_all = np.ascontiguousarray(
        hW.transpose(2, 0, 1).reshape(2, 128, 4, NUM_CLASSES).transpose(1, 0, 2, 3)
    ).astype(np.float32)                                          # [128,2,4,3]
    headb_all = np.ascontiguousarray(
        inputs["head_b"].astype(np.float32).T.reshape(NUM_CLASSES, 4)
    )

    sel = np.zeros((8, NSUB * 128), np.float16)
    for g in range(NSUB):
        sel[g % 8, g * 128:(g + 1) * 128] = 1.0

    return dict(
        W1_all=to_chunks(W1, np.float16),
        Wt_all=to_chunks(Wt, np.float16),
        PtT_all=PtT_all,
        Dp_all=Dp_all,
        pbt_all=pbt_all,
        corrW_row=corrW_row,
        sel_all=sel,
        ident_in=np.ascontiguousarray(np.eye(128, dtype=np.float16)),
        onesI_in=np.ascontiguousarray(
            np.tile(np.eye(8, dtype=np.float16)[None] / D_MODEL, (128, 1, 1))
        ),
        headWT_all=headWT_all,
        headb_all=headb_all,
    )


def _split_drain_waits(nc, mybir, maxw=1):
    """Hoist excess sync waits onto same-engine NOPs (walrus ISA limit)."""
    for f in nc.m.functions:
        for blk in f.blocks:
            insts = list(blk.instructions)
            changed = False
            new_list = []
            for ins in insts:
                w = (
                    list(ins.sync_info.on_wait)
                    if ins.sync_info and ins.sync_info.on_wait
                    else []
                )
                if len(w) > maxw:
                    changed = True
                    extra, keep = w[:-maxw], w[-maxw:]
                    for j in range(0, len(extra), maxw):
                        nop = mybir.InstNoOp(
                            name=f"{ins.name}-wsplit{j}", ins=[], outs=[]
                        )
                        nop.engine = ins.engine
                        nop.sync_info = mybir.SyncInfo(
                            on_wait=extra[j : j + maxw], on_update=[]
                        )
                        new_list.append(nop)
                    ins.sync_info.on_wait = keep
                new_list.append(ins)
            if changed:
                blk.instructions = new_list


def _build_nc(sim_safe=False, split=True, cfg=None):
    import concourse.bass as bass
    import concourse.tile as tile
    import concourse.mybir as mybir

    F32 = mybir.dt.float32
    F16 = mybir.dt.float16
    OP = mybir.AluOpType
    ACTF = mybir.ActivationFunctionType
    GELU = ACTF.Sigmoid if sim_safe else ACTF.Gelu
    if cfg is None:
        cfg = CONFIG

    nc = bass.Bass("TRN2", target_bir_lowering=False, debug=False)

    d_h0 = nc.dram_tensor("h0_in", [128, 2, SEQ], F16, kind="ExternalInput")
    d_W1 = nc.dram_tensor("W1_all", [128, N_LAYERS, 2, JW], F16, kind="ExternalInput")
    d_Wt = nc.dram_tensor("Wt_all", [128, N_LAYERS, 2, TAIL], F16, kind="ExternalInput")
    d_Pt = nc.dram_tensor("PtT_all", [128, N_LAYERS, 2, D_MODEL], F16, kind="ExternalInput")
    d_Dp = nc.dram_tensor("Dp_all", [128, N_LAYERS, 2], F32, kind="ExternalInput")
    d_pbt = nc.dram_tensor("pbt_all", [128, N_LAYERS, 2], F32, kind="ExternalInput")
    d_corrW = nc.dram_tensor("corrW_row", [1, D_MODEL], F16, kind="ExternalInput")
    d_x16 = nc.dram_tensor("x_row16", [1, SEQ], F16, kind="ExternalInput")
    d_ident = nc.dram_tensor("ident_in", [128, 128], F16, kind="ExternalInput")
    d_sel = nc.dram_tensor("sel_all", [8, NSUB * 128], F16, kind="ExternalInput")
    d_onesI = nc.dram_tensor("onesI_in", [128, 8, 8], F16, kind="ExternalInput")
    d_hW = nc.dram_tensor("headWT_all", [128, 2, 4, NUM_CLASSES], F32, kind="ExternalInput")
    d_hb = nc.dram_tensor("headb_all", [NUM_CLASSES, 4], F32, kind="ExternalInput")
    d_out = nc.dram_tensor("logits_out", [NUM_CLASSES, 4], F32, kind="ExternalOutput")

    with tile.TileContext(nc) as tc:
        from contextlib import ExitStack

        ctx = ExitStack()
        with ctx:
            const = ctx.enter_context(tc.tile_pool(name="const", bufs=1))
            stream = ctx.enter_context(tc.tile_pool(name="stream", bufs=cfg.get("stream_bufs", 3)))
            ypool = ctx.enter_context(tc.tile_pool(name="ypool", bufs=cfg.get("y_bufs", 2)))
            vpool = ctx.enter_context(tc.tile_pool(name="vpool", bufs=cfg.get("v_bufs", 2)))
            sqpool = ctx.enter_context(tc.tile_pool(name="sqpool", bufs=cfg.get("sq_bufs", 2)))
            epool = ctx.enter_context(tc.tile_pool(name="epool", bufs=cfg.get("e_bufs", 2)))
            small = ctx.enter_context(tc.tile_pool(name="small", bufs=cfg.get("small_bufs", 2)))
            stat = ctx.enter_context(tc.tile_pool(name="stat", bufs=cfg.get("stat_bufs", 2)))
            pacc = ctx.enter_context(tc.tile_pool(name="pacc", bufs=5))
            pw = ctx.enter_context(tc.tile_pool(name="pw", bufs=cfg.get("pw_bufs", 4), space="PSUM"))
            pstat = ctx.enter_context(tc.tile_pool(name="pstat", bufs=1, space="PSUM"))
            pinv = ctx.enter_context(tc.tile_pool(name="pinv", bufs=cfg.get("pinv_bufs", 2), space="PSUM"))

            # ---- constants / weights to SBUF ----
            # DMA staging split across three issue queues so layer 0's
            # working set lands in parallel: SP streams h0 + conv weights,
            # ACT's queue takes the gelu/proj-critical constants, DVE's
            # queue takes the tail tables and cold layers.
            t = stream.tile([128, 2, SEQ], F16, tag="stream")
            Dp_sb = const.tile([128, N_LAYERS, 2], F32)
            Pt_sb = const.tile([128, N_LAYERS, 2, D_MODEL], F16)
            W1_sb = const.tile([128, N_LAYERS, 2, JW], F16)
            Wt_sb = const.tile([128, N_LAYERS, 2, TAIL], F16)
            pbt_sb = const.tile([128, N_LAYERS, 2], F32)
            x16_row = const.tile([1, SEQ], F16)
            corrW_row = const.tile([1, D_MODEL], F16)
            onesI_sb = const.tile([128, 8, 8], F16)
            ident_sb = const.tile([128, 128], F16)
            sel_sb = const.tile([8, NSUB * 128], F16)
            hW_sb = const.tile([128, 2, 4, NUM_CLASSES], F32)
            hb_sb = const.tile([NUM_CLASSES, 4], F32)

            nc.sync.dma_start(out=t[:, :, 0:NTW], in_=d_h0.ap()[:, :, 0:NTW])
            nc.sync.dma_start(out=Dp_sb[:], in_=d_Dp.ap())
            nc.sync.dma_start(out=W1_sb[:, 0], in_=d_W1.ap()[:, 0])
            nc.sync.dma_start(out=t[:, :, NTW : 2 * NTW], in_=d_h0.ap()[:, :, NTW : 2 * NTW])
            nc.sync.dma_start(out=Wt_sb[:, 0], in_=d_Wt.ap()[:, 0])
            nc.sync.dma_start(out=Pt_sb[:, 0], in_=d_Pt.ap()[:, 0])
            nc.sync.dma_start(out=t[:, :, 2 * NTW : 3 * NTW], in_=d_h0.ap()[:, :, 2 * NTW : 3 * NTW])
            nc.sync.dma_start(out=t[:, :, 3 * NTW :], in_=d_h0.ap()[:, :, 3 * NTW :])
            nc.sync.dma_start(out=pbt_sb[:], in_=d_pbt.ap())
            nc.sync.dma_start(out=x16_row[:], in_=d_x16.ap())
            nc.sync.dma_start(out=corrW_row[:], in_=d_corrW.ap())
            nc.sync.dma_start(out=onesI_sb[:], in_=d_onesI.ap())
            nc.sync.dma_start(out=ident_sb[:], in_=d_ident.ap())
            nc.sync.dma_start(out=sel_sb[:], in_=d_sel.ap())
            for _i in range(1, N_LAYERS):
                nc.sync.dma_start(out=W1_sb[:, _i], in_=d_W1.ap()[:, _i])
                nc.sync.dma_start(out=Wt_sb[:, _i], in_=d_Wt.ap()[:, _i])
                nc.sync.dma_start(out=Pt_sb[:, _i], in_=d_Pt.ap()[:, _i])
            nc.sync.dma_start(out=hW_sb[:], in_=d_hW.ap())
            nc.sync.dma_start(out=hb_sb[:], in_=d_hb.ap())
            eps_col = const.tile([8, 1], F32)
            nc.vector.memset(eps_col, LN_EPS)
            # eps injection for the stats PSUM accumulation: a [1,8] x [1,128]
            # rank-1 matmul adding eps to every stat cell.  1e-5 is subnormal
            # in fp16, so split it 2.56e-3 * 3.90625e-3.
            epsT_row = const.tile([1, 8], F16)
            nc.vector.memset(epsT_row, LN_EPS * 256.0)
            ones_row = const.tile([1, 128], F16)
            nc.vector.memset(ones_row, 1.0 / 256.0)

            logits_sb = const.tile([NUM_CLASSES, 4], F32)
            pooled = {}

            # PE p-state warmup: ~3us of junk matmuls overlapping the DMA
            # staging so layer 0's proj matmuls run at full clock.
            if cfg.get("pe_warmup", 60):
                warm = pstat.tile([8, 128], F32, tag="pstat")
                for _ in range(cfg.get("pe_warmup", 60)):
                    nc.tensor.matmul(
                        warm[:], lhsT=epsT_row[:], rhs=ones_row[:],
                        start=True, stop=True,
                    )

            # ---------------- shared tail: Y -> t_next ----------------
            def tail(Y, sq, li, exit_idx, ps_tiles):
                """Y fp16 [128,2,SEQ] (centered pre-LN) -> t fp16 (normalized).

                stats (tiny PE matmuls into per-pair PSUM stat tiles) ->
                rsqrt (ACT eps-evac, DVE reciprocal_approx_fast, ACT Sqrt)
                -> PE selector-broadcast -> evac -> t' STT (4x, accum_out
                doubles as exit-layer pooled partial sums)."""
                S_psA = pstat.tile([8, 128], F32, tag="pstat")
                S_psB = pstat.tile([8, 128], F32, tag="pstatB")
                S_psp = {0: S_psA, 1: S_psB}
                t_new = stream.tile([128, 2, SEQ], F16, tag="stream")
                r16p = {}
                ib_pair = {}

                PR = cfg.get("psum_recip")

                def sq_chunk(nt):
                    # emitted HERE (not in the proj loop) so pair-0's recip
                    # isn't queued behind pair-1's sq in DVE program order
                    sl = slice(nt * NTW, (nt + 1) * NTW)
                    se = cfg["sq_eng"][nt]
                    if se == "P":
                        # ident chunks: ps already holds ps+t, so Square can
                        # read PSUM with the pbt bias — no join dependency
                        for m in range(2):
                            nc.scalar.activation(
                                out=sq[:, m, sl], in_=ps_tiles[nt, m][:],
                                func=ACTF.Square,
                                bias=pbt_sb[:, li, m : m + 1], scale=1.0,
                            )
                        return
                    if se == "d" and cfg.get("sq_merge"):
                        nc.vector.tensor_tensor(
                            out=sq[:, :, sl], in0=Y[:, :, sl],
                            in1=Y[:, :, sl], op=OP.mult,
                        )
                        return
                    for m in range(2):
                        if se == "a":
                            nc.scalar.activation(
                                out=sq[:, m, sl], in_=Y[:, m, sl],
                                func=ACTF.Square, bias=0.0, scale=1.0,
                            )
                        elif se == "p":
                            nc.gpsimd.tensor_tensor(
                                out=sq[:, m, sl], in0=Y[:, m, sl],
                                in1=Y[:, m, sl], op=OP.mult,
                            )
                        else:
                            nc.vector.tensor_tensor(
                                out=sq[:, m, sl], in0=Y[:, m, sl],
                                in1=Y[:, m, sl], op=OP.mult,
                            )

                def stats_chunk(nt):
                    sq_chunk(nt)
                    if PR and nt % 2 == 0:
                        # open the pair's accumulation group with the eps
                        # rank-1 term so the DVE reciprocal can read PSUM raw
                        nc.tensor.matmul(
                            S_psp[nt // 2][:],
                            lhsT=epsT_row[:],
                            rhs=ones_row[:],
                            start=True, stop=False,
                        )
                    for c in range(4):
                        g = nt * 4 + c
                        csl = slice(g * 128, (g + 1) * 128)
                        for m in range(2):
                            nc.tensor.matmul(
                                S_psp[g // 8][:],
                                lhsT=onesI_sb[:, g % 8, :],
                                rhs=sq[:, m, csl],
                                start=(not PR) and (g % 8 == 0 and m == 0),
                                stop=(g % 8 == 7 and m == 1),
                            )

                def rsqrt_pair(p):
                    # inv = 1/sqrt(S+eps).  psum_recip: eps was accumulated by
                    # PE, so DVE reciprocal_approx_fast (custom op, ~18 bits)
                    # reads the PSUM stat tile directly and ACT Sqrt finishes
                    # straight to fp16 — two ops, no separate eps/convert.
                    y32 = stat.tile([8, 128], F32, tag="y32")
                    r16 = stat.tile([8, 128], F16, tag="r16")
                    r16p[p] = r16
                    if PR:
                        nc.vector.reciprocal(out=y32[:], in_=S_psp[p][:])
                        nc.scalar.activation(
                            out=r16[:], in_=y32[:], func=ACTF.Sqrt,
                            bias=0.0, scale=1.0,
                        )
                        return
                    v16 = stat.tile([8, 128], F32, tag="v16")
                    nc.scalar.activation(
                        out=v16[:], in_=S_psp[p][:], func=ACTF.Sqrt,
                        bias=eps_col[:], scale=1.0,
                    )
                    nc.vector.reciprocal(out=y32[:], in_=v16[:])
                    if cfg.get("rconv_eng", ("a", "a"))[p] == "a":
                        nc.scalar.activation(
                            out=r16[:], in_=y32[:], func=ACTF.Copy,
                            bias=0.0, scale=1.0,
                        )
                    else:
                        nc.vector.tensor_scalar(
                            out=r16[:], in0=y32[:], scalar1=0.0,
                            scalar2=None, op0=OP.add,
                        )

                pc = None
                if exit_idx is not None:
                    pc = pacc.tile([128, 2, NT], F32, tag="pacc")
                    pooled[exit_idx] = pc

                last = li == N_LAYERS - 1

                def bcast_tp(nt):
                    # reads only this nt-pair's r16 tile so pair-0 broadcasts
                    # don't wait on pair-1's rsqrt.  evac_pair: both chunks of
                    # the pair share one [128, 2*NTW] ib tile and a single
                    # [1024]-wide evac emitted with the odd chunk.
                    sl = slice(nt * NTW, (nt + 1) * NTW)
                    r16 = r16p[nt // 2]
                    EP = cfg.get("evac_pair")
                    if EP:
                        if nt % 2 == 0:
                            ib2 = pinv.tile([128, 2 * NTW], F32, tag="pinv")
                            ib_pair[nt // 2] = ib2
                            ib = ib2[:, 0:NTW]
                        else:
                            ib2 = ib_pair[nt // 2]
                            ib = ib2[:, NTW : 2 * NTW]
                    else:
                        ib = pinv.tile([128, NTW], F32, tag="pinv")
                    for c in range(4):
                        g = nt * 4 + c
                        nc.tensor.matmul(
                            ib[:, c * 128 : (c + 1) * 128],
                            lhsT=sel_sb[:, g * 128 : (g + 1) * 128],
                            rhs=r16[:],
                            start=True, stop=True,
                        )
                    ev = cfg["evac_eng"][nt]
                    if EP and ev != "D":
                        # single evac + t' for both chunks, on the odd nt
                        if nt % 2 == 0:
                            return
                        i16w = epool.tile([128, 1, 2 * NTW], F16, tag="i16")
                        if ev == "a":
                            nc.scalar.activation(
                                out=i16w[:, 0], in_=ib2[:], func=ACTF.Copy,
                                bias=0.0, scale=1.0,
                            )
                        else:
                            nc.vector.tensor_scalar(
                                out=i16w[:, 0], in0=ib2[:], scalar1=0.0,
                                scalar2=None, op0=OP.add,
                            )
                        for snt in (nt - 1, nt):
                            ssl = slice(snt * NTW, (snt + 1) * NTW)
                            ioff = (snt % 2) * NTW
                            tp = cfg.get("tp_eng", ("d",) * 4)[snt]
                            if last:
                                tp = cfg.get("last_tp", "d")
                            for m in range(2):
                                if tp == "p":
                                    nc.gpsimd.tensor_tensor(
                                        out=t_new[:, m, ssl], in0=Y[:, m, ssl],
                                        in1=i16w[:, 0, ioff : ioff + NTW], op=OP.mult,
                                    )
                                else:
                                    nc.vector.tensor_tensor(
                                        out=t_new[:, m, ssl], in0=Y[:, m, ssl],
                                        in1=i16w[:, 0, ioff : ioff + NTW], op=OP.mult,
                                    )
                            if pc is not None:
                                for m in range(2):
                                    nc.vector.tensor_scalar(
                                        out=sq[:, m, ssl], in0=t_new[:, m, ssl],
                                        scalar1=0.0, scalar2=0.0, op0=OP.add,
                                        op1=OP.add,
                                        accum_out=pc[:, m, snt : snt + 1],
                                    )
                        return

                    def tprime():
                        tp = cfg.get("tp_eng", ("d",) * 4)[nt]
                        if last:
                            tp = cfg.get("last_tp", "d")
                        for m in range(2):
                            if tp == "p":
                                nc.gpsimd.tensor_tensor(
                                    out=t_new[:, m, sl], in0=Y[:, m, sl],
                                    in1=i16[:, 0], op=OP.mult,
                                )
                            else:
                                nc.vector.tensor_tensor(
                                    out=t_new[:, m, sl], in0=Y[:, m, sl],
                                    in1=i16[:, 0], op=OP.mult,
                                )

                    def pooled_nt():
                        # exit layers: per-chunk pooled accum (4x TS) right
                        # after the chunk's t', so the epilogue pipelines; the
                        # dead sq tile is the throwaway elementwise output.
                        if pc is None:
                            return
                        for m in range(2):
                            nc.vector.tensor_scalar(
                                out=sq[:, m, sl], in0=t_new[:, m, sl],
                                scalar1=0.0, scalar2=0.0, op0=OP.add,
                                op1=OP.add,
                                accum_out=pc[:, m, nt : nt + 1],
                            )

                    if ev == "D":
                        # no evac: t' STT reads ib from PSUM at fp32 rate
                        for m in range(2):
                            nc.vector.scalar_tensor_tensor(
                                out=t_new[:, m, sl], in0=Y[:, m, sl],
                                scalar=0.0, in1=ib[:],
                                op0=OP.bypass, op1=OP.mult,
                            )
                        pooled_nt()
                        return
                    i16 = epool.tile([128, 1, NTW], F16, tag="i16")
                    if ev == "a":
                        nc.scalar.activation(
                            out=i16[:, 0], in_=ib[:], func=ACTF.Copy,
                            bias=0.0, scale=1.0,
                        )
                    elif ev == "d":
                        nc.vector.tensor_scalar(
                            out=i16[:, 0], in0=ib[:], scalar1=0.0,
                            scalar2=None, op0=OP.add,
                        )
                    else:  # 'p'
                        nc.gpsimd.tensor_scalar(
                            out=i16[:, 0], in0=ib[:], scalar1=0.0,
                            scalar2=None, op0=OP.add,
                        )
                    tprime()
                    pooled_nt()

                # part A: everything pair-0 needs, plus pair-1's stats.
                if cfg.get("tp_first"):
                    stats_chunk(0)
                    stats_chunk(1)
                    rsqrt_pair(0)
                    bcast_tp(0)
                    bcast_tp(1)
                    stats_chunk(2)
                    stats_chunk(3)
                else:
                    stats_chunk(0)
                    stats_chunk(1)
                    rsqrt_pair(0)
                    stats_chunk(2)
                    stats_chunk(3)
                    bcast_tp(0)
                    bcast_tp(1)

                def finish():
                    # pair-1 tail — deferrable past the next layer's head so
                    # its ops don't block gelu/proj in engine program order.
                    rsqrt_pair(1)
                    bcast_tp(2)
                    bcast_tp(3)
                    if exit_idx is not None:
                        emit_head(exit_idx)

                return t_new, finish

            def emit_head(e):
                pc = pooled[e]  # [128, 2, nacc] f32 chunk partial sums
                pr = pacc.tile([128, 2], F32, tag=f"pr{e}")
                nc.vector.tensor_tensor(
                    out=pr[:], in0=pc[:, :, 0], in1=pc[:, :, 1], op=OP.add
                )
                if pc.shape[2] == 4:
                    p23 = pacc.tile([128, 2], F32, tag=f"p23{e}")
                    nc.vector.tensor_tensor(
                        out=p23[:], in0=pc[:, :, 2], in1=pc[:, :, 3], op=OP.add
                    )
                    nc.vector.tensor_tensor(
                        out=pr[:], in0=pr[:], in1=p23[:], op=OP.add
                    )
                pl = pstat.tile([NUM_CLASSES, 1], F32, tag="pstat")
                for k in range(2):
                    nc.tensor.matmul(
                        pl[:],
                        lhsT=hW_sb[:, k, e],
                        rhs=pr[:, k : k + 1],
                        start=(k == 0),
                        stop=(k == 1),
                    )
                nc.vector.tensor_scalar(
                    out=logits_sb[:, e : e + 1], in0=pl[:],
                    scalar1=hb_sb[:, e : e + 1], scalar2=None, op0=OP.add,
                )

            # layer-0 stream t0 = h0 arrives pre-computed via the first DMA

            # ---------------- layers ----------------
            # Emission is interleaved across layers: layer i-1's pair-1 tail
            # (finish_prev) is emitted AFTER layer i's first gelu piece and
            # nt=0 proj/join, so the new layer's head isn't queued behind the
            # old layer's tail in ACT/PE/DVE program order.
            finish_prev = None
            for i in range(N_LAYERS):
                v = vpool.tile([128, 2, SEQ], F16, tag="v")
                sinf = small.tile([128, 2], F32, tag="sinf")
                sscr = small.tile([128, 2, JW], F16, tag="sscr")
                conv = small.tile([128, 2, TAIL], F16, tag="conv")
                ut = small.tile([128, 2, TAIL], F16, tag="ut")
                conv_eng = nc.gpsimd if cfg.get("conv_pool") else nc.vector
                for m in range(2):
                    conv_eng.scalar_tensor_tensor(
                        out=sscr[:, m], in0=t[:, m, 0:JW], scalar=0.0,
                        in1=W1_sb[:, i, m], op0=OP.bypass, op1=OP.mult,
                        accum_out=sinf[:, m : m + 1],
                    )
                    conv_eng.tensor_scalar(
                        out=conv[:, m], in0=Wt_sb[:, i, m],
                        scalar1=sinf[:, m : m + 1], scalar2=None,
                        op0=OP.mult,
                    )

                pieces = cfg.get("gelu_pieces")
                assert len(pieces) >= 3 and pieces[0][1] >= NTW and pieces[1][1] >= 2 * NTW

                def gelu_piece(k, i=i, v=v):
                    g0, g1 = pieces[k]
                    for m in range(2):
                        nc.scalar.activation(
                            out=v[:, m, g0:g1], in_=t[:, m, g0:g1],
                            func=GELU, bias=0.0,
                            scale=Dp_sb[:, i, m : m + 1],
                        )

                Y = ypool.tile([128, 2, SEQ], F16, tag="ypool")
                sq = sqpool.tile([128, 2, SEQ], F16, tag="sq")
                ps_tiles = {}

                def proj_join(nt, i=i, v=v, Y=Y, t=t, ps_tiles=ps_tiles):
                    sl = slice(nt * NTW, (nt + 1) * NTW)
                    jcfg = cfg["join_eng"]
                    ident = cfg.get("ident_nt", (0, 0, 0, 0))[nt]
                    for m in range(2):
                        je = jcfg[nt * 2 + m] if len(jcfg) == 8 else jcfg[nt]
                        ps = pw.tile([128, NTW], F32, tag="pw")
                        ps_tiles[nt, m] = ps
                        if ident:
                            nc.tensor.matmul(
                                ps[:], lhsT=ident_sb[:], rhs=t[:, m, sl],
                                start=True, stop=False,
                            )
                        for k in range(2):
                            nc.tensor.matmul(
                                ps[:],
                                lhsT=Pt_sb[:, i, k, m * 128 : (m + 1) * 128],
                                rhs=v[:, k, sl],
                                start=(k == 0) and not ident,
                                stop=(k == 1) and i != 0,
                            )
                        if i == 0:
                            # center layer-0's x-dependent channel mean
                            nc.tensor.matmul(
                                ps[:],
                                lhsT=corrW_row[:, m * 128 : (m + 1) * 128],
                                rhs=x16_row[:, sl],
                                start=False, stop=True,
                            )
                        if ident:
                            # +t already in PSUM: the join is a pure bias evac
                            if je == "a":
                                nc.scalar.activation(
                                    out=Y[:, m, sl], in_=ps[:], func=ACTF.Identity,
                                    bias=pbt_sb[:, i, m : m + 1], scale=1.0,
                                )
                            elif je == "p":
                                nc.gpsimd.tensor_scalar(
                                    out=Y[:, m, sl], in0=ps[:],
                                    scalar1=pbt_sb[:, i, m : m + 1], scalar2=None,
                                    op0=OP.add,
                                )
                            else:
                                nc.vector.tensor_scalar(
                                    out=Y[:, m, sl], in0=ps[:],
                                    scalar1=pbt_sb[:, i, m : m + 1], scalar2=None,
                                    op0=OP.add,
                                )
                        elif je in ("a", "A"):
                            w16 = epool.tile([128, NTW], F16, tag="w16")
                            nc.scalar.activation(
                                out=w16[:], in_=ps[:], func=ACTF.Identity,
                                bias=pbt_sb[:, i, m : m + 1], scale=1.0,
                            )
                            if je == "A":  # SBUF add half on Pool
                                nc.gpsimd.tensor_tensor(
                                    out=Y[:, m, sl], in0=w16[:],
                                    in1=t[:, m, sl], op=OP.add,
                                )
                            else:
                                nc.vector.tensor_tensor(
                                    out=Y[:, m, sl], in0=w16[:],
                                    in1=t[:, m, sl], op=OP.add,
                                )
                        else:  # 'd'
                            nc.vector.scalar_tensor_tensor(
                                out=Y[:, m, sl], in0=ps[:],
                                scalar=pbt_sb[:, i, m : m + 1], in1=t[:, m, sl],
                                op0=OP.add, op1=OP.add,
                            )

                gelu_piece(0)
                proj_join(0)
                if finish_prev is not None and cfg.get("defer", True):
                    finish_prev()
                    finish_prev = None
                gelu_piece(1)
                proj_join(1)
                for k in range(2, len(pieces)):
                    gelu_piece(k)
                for m in range(2):
                    conv_eng.scalar_tensor_tensor(
                        out=ut[:, m], in0=t[:, m, SEQ - TAIL :],
                        scalar=Dp_sb[:, i, m : m + 1], in1=conv[:, m],
                        op0=OP.mult, op1=OP.add,
                    )
                    nc.scalar.activation(
                        out=v[:, m, SEQ - TAIL :], in_=ut[:, m],
                        func=GELU, bias=0.0, scale=1.0,
                    )
                proj_join(2)
                proj_join(3)

                exit_idx = EXIT_LAYERS.index(i) if i in EXIT_LAYERS else None
                t, fin = tail(Y, sq, i, exit_idx, ps_tiles)
                if finish_prev is not None:
                    finish_prev()
                finish_prev = fin

            finish_prev()  # last layer's tail also emits head 3
            nc.sync.dma_start(out=d_out.ap(), in_=logits_sb[:])

    if split:
        _split_drain_waits(nc, mybir)
    return nc


def _forward_fallback(inputs):
    """Numpy-only exact reference computation (general-inputs path)."""
    import math

    erf = np.vectorize(math.erf)
    x = inputs["x"].astype(np.float32)
    h = x[:, :, 0:1] * inputs["in_W"][None, None, :, 0] + inputs["in_b"]
    logits = []
    head = 0
    Lf = np.arange(SEQ, dtype=np.float32)
    for i in range(N_LAYERS):
        A = 1.0 / (1.0 + np.exp(-inputs["A_params"][i].astype(np.float32)))
        K = (
            inputs["C_params"][i][:, None]
            * (A[:, None] ** Lf[None, :])
            * inputs["B_params"][i][:, None]
        ).astype(np.float32)
        ht = np.swapaxes(h, 1, 2).astype(np.float32)
        out = np.empty_like(ht)
        for b in range(x.shape[0]):
            for d in range(D_MODEL):
                c = np.correlate(
                    np.concatenate([np.zeros(SEQ - 1, np.float32), ht[b, d]]),
                    K[d][::-1],
                    mode="valid",
                )
                out[b, d] = c[:SEQ]
        out = out + inputs["D_params"][i][None, :, None] * ht
        u = np.swapaxes(out, 1, 2)
        vg = u * 0.5 * (1.0 + erf(u / np.sqrt(2.0)))
        w = vg.astype(np.float32) @ inputs["proj_W"][i].T + inputs["proj_b"][i]
        y = h + w
        mu = y.mean(-1, keepdims=True)
        var = y.var(-1, keepdims=True)
        h = (y - mu) / np.sqrt(var + LN_EPS) * inputs["ln_g"][i] + inputs["ln_b"][i]
        if i in EXIT_LAYERS:
            pooled = h.mean(axis=1)
            logits.append(pooled @ inputs["head_W"][head].T + inputs["head_b"][head])
            head += 1
    return np.stack(logits, 0).astype(np.float32)


def _run_device(inputs, trace=False):
    from concourse import bass_utils

    key = "nc"
    if key not in _CACHE:
        _CACHE[key] = _build_nc(sim_safe=False)
    nc = _CACHE[key]

    weights = _host_prep(inputs)
    x = np.asarray(inputs["x"], dtype=np.float32)
    inWf = np.asarray(inputs["in_W"], dtype=np.float64)[:, 0]
    inbf = np.asarray(inputs["in_b"], dtype=np.float64)
    in_maps = []
    for b in range(BATCH):
        m = dict(weights)
        xb = x[b, :, 0].astype(np.float64)
        h0 = xb[:, None] * inWf[None, :] + inbf          # [SEQ, 256]
        m["h0_in"] = np.ascontiguousarray(
            h0.reshape(SEQ, 2, 128).transpose(2, 1, 0)
        ).astype(np.float16)                              # [128, 2, SEQ]
        m["x_row16"] = x[b, :, 0].reshape(1, SEQ).astype(np.float16)
        in_maps.append(m)
    res = bass_utils.run_bass_kernel_spmd(
        nc, in_maps, core_ids=list(range(BATCH)), trace=trace
    )
    out = np.empty((4, BATCH, NUM_CLASSES), dtype=np.float32)
    for b in range(BATCH):
        lg = res.results[b]["logits_out"]
        out[:, b, :] = lg.T
    return out, res


def kernel(**inputs):
    inputs = {k: np.asarray(v) for k, v in inputs.items()}
    maxA = float(1.0 / (1.0 + np.exp(-np.abs(inputs["A_params"]).max())))
    fast = (
        np.all(inputs["ln_g"] == 1.0)
        and np.all(inputs["ln_b"] == 0.0)
        and maxA**TAIL < 1e-30
        and inputs["x"].shape == (BATCH, SEQ, 1)
    )
    if not fast:
        return _forward_fallback(inputs)
    out, _ = _run_device(inputs, trace=False)
    return out
